# revision 40
# baseline (speedup 1.0000x reference)
"""Multi-head attention on 8 Trainium2 NeuronCores.

Sharding: data-parallel over batch (2) x tensor-parallel over heads
(16 heads -> 4 per core). Each core computes QKV projections for its
4 heads, masked softmax attention, and a partial output projection
(row-parallel Wo); the host sums the 4 per-batch partials and folds
in the biases that commute with the linear output projection
(out[b] = sum_partials.T + bo + Wo @ bv).  bk is dropped entirely:
adding bk to K only shifts every query's scores by a per-query
constant, which softmax cancels exactly.

Fully software-pipelined single-pass schedule (v2):
  - ScalarE runs ONLY the softmax exps (the pacer: 2x ~1.15us per key
    block); PSUM drains on DVE, denominator broadcast via SBUF->SBUF
    DMA with a 0-stride partition AP (no GpSimd, no staging copies).
  - The (qg, kb) loops are flattened into one global slot stream so
    the next query group's score matmuls sit in the PE queue BEFORE
    the previous group's last PV/den matmuls (which wait on exp).
    This removes the 3.5us PE bubble per query-group boundary that
    also caused HAM re-throttling.
  - kb_pv orders all 4 den (ones) matmuls after both PV pairs so the
    4 den matmuls run concurrently in 4 distinct PE column strips.
  - Normalize works on head PAIRS: X psum banks are drained with two
    [128,512] copies (heads share a bank), one reciprocal, two
    [128,512] multiplies -- half the DVE ops of per-head normalize.
  - Output projection of query group qg is interleaved into query
    group qg+1's key loop; out is fp16, host accumulates in fp32.
"""

import os
import numpy as np

B, S, D = 2, 2048, 1024
H, DK = 16, 64
NCORES = 8
CPB = 4                 # cores per batch
HPC = H // CPB          # 4 heads per core
HD = HPC * DK           # 256
P = 128
QG = 512                # query block
NQG = S // QG
NKB = S // P            # key blocks
NDC = D // P            # d_in chunks
SCALE = 1.0 / np.sqrt(DK)

_CACHE = {}


def _build_nc():
    import concourse.mybir as mybir
    import concourse.tile as tile
    from concourse import bacc, bass

    f32 = mybir.dt.float32
    f16 = mybir.dt.float16
    AF = mybir.ActivationFunctionType
    OP = mybir.AluOpType

    nc = bacc.Bacc("TRN2", target_bir_lowering=False, debug=False)
    xqT = nc.dram_tensor("xqT", [D, S], f16, kind="ExternalInput")
    xkT = nc.dram_tensor("xkT", [D, S], f16, kind="ExternalInput")
    xvT = nc.dram_tensor("xvT", [D, S], f16, kind="ExternalInput")
    wqT = nc.dram_tensor("wqT", [D, HD], f16, kind="ExternalInput")
    wkT = nc.dram_tensor("wkT", [D, HD], f16, kind="ExternalInput")
    wvT = nc.dram_tensor("wvT", [D, HD], f16, kind="ExternalInput")
    woT = nc.dram_tensor("woT", [HD, D], f16, kind="ExternalInput")
    bq = nc.dram_tensor("bq", [HD, 1], f32, kind="ExternalInput")
    maskT = nc.dram_tensor("maskT", [S, S], f16, kind="ExternalInput")
    outT = nc.dram_tensor("outT", [D, S], f16, kind="ExternalOutput")

    with tile.TileContext(nc) as tc:
        with (
            tc.tile_pool(name="wpool", bufs=1) as wpool,
            tc.tile_pool(name="xkp", bufs=2) as xkp,
            tc.tile_pool(name="xqp", bufs=2) as xqp,
            tc.tile_pool(name="xvp", bufs=2) as xvp,
            tc.tile_pool(name="proj", bufs=1) as proj,
            tc.tile_pool(name="vpool", bufs=1) as vpool,
            tc.tile_pool(name="mpool", bufs=4) as mpool,
            tc.tile_pool(name="ppool", bufs=4) as ppool,
            tc.tile_pool(name="xspool", bufs=2) as xspool,
            tc.tile_pool(name="rcpool", bufs=2) as rcpool,
            tc.tile_pool(name="rbpool", bufs=2) as rbpool,
            tc.tile_pool(name="xnpool", bufs=1) as xnpool,
            tc.tile_pool(name="ostpool", bufs=3) as ostpool,
        ):
            # ---------------- static SBUF tensors ----------------
            wq_sb = wpool.tile([P, NDC * HD], f16, tag="wq")
            wk_sb = wpool.tile([P, NDC * HD], f16, tag="wk")
            wv_sb = wpool.tile([P, NDC * HD], f16, tag="wv")
            wo_sb = [wpool.tile([P, D], f16, tag=f"wo{i}", name=f"wo{i}")
                     for i in range(2)]
            bq_sb = [wpool.tile([P, 1], f32, tag=f"bq{i}", name=f"bqt{i}")
                     for i in range(2)]
            ones_sb = wpool.tile([P, 1], f16, tag="ones")
            ones64 = wpool.tile([P, DK], f16, tag="ones64")

            KT = [proj.tile([P, S], f16, tag=f"KT{i}", name=f"KT{i}")
                  for i in range(2)]
            QT = [proj.tile([P, S], f16, tag=f"QT{i}", name=f"QT{i}")
                  for i in range(2)]
            Vt = [vpool.tile([P, HD], f16, tag=f"V{t}", name=f"Vt{t}")
                  for t in range(NKB)]
            XN = [xnpool.tile([P, S], f16, tag=f"XN{i}", name=f"XN{i}")
                  for i in range(2)]

            def _load_w(wsb, wdr):
                nc.sync.dma_start(
                    wsb[:].rearrange("p (c n) -> p c n", n=HD),
                    wdr.rearrange("(c p) n -> p c n", p=P),
                )

            def _xslice(xdr):
                return xdr.rearrange("(c p) n -> p c n", p=P)

            xk_t, xq_t, xv_t = {}, {}, {}

            def dma_x(pool, store, xdr, tg, nm):
                t = pool.tile([P, NDC, QG], f16, tag=nm, name=f"{nm}{tg}")
                nc.sync.dma_start(
                    t[:], _xslice(xdr)[:, :, tg * QG:(tg + 1) * QG])
                store[tg] = t

            with (
                tc.tile_pool(name="scps", bufs=2, space="PSUM") as scps,
                tc.tile_pool(name="xps", bufs=1, space="PSUM") as xps,
                tc.tile_pool(name="denps", bufs=1, space="PSUM") as denps,
                tc.tile_pool(name="auxps", bufs=1, space="PSUM") as auxps,
            ):
                # ---------- projection helpers ----------
                def kq_mm_full(ps, wsb, xt):
                    """both kc halves of one token group into ps [P, 2*QG]"""
                    for c in range(NDC):
                        for kc in range(2):
                            nc.tensor.matmul(
                                ps[:, kc * QG:(kc + 1) * QG],
                                wsb[:, c * HD + kc * P:c * HD + (kc + 1) * P],
                                xt[:, c, :],
                                start=(c == 0), stop=(c == NDC - 1),
                            )

                def k_drain(src_ap, tg, kc):
                    nc.vector.tensor_copy(
                        KT[kc][:, tg * QG:(tg + 1) * QG], src_ap)

                def q_drain(src_ap, tg, kc):
                    nc.vector.tensor_scalar(
                        QT[kc][:, tg * QG:(tg + 1) * QG],
                        src_ap,
                        bq_sb[kc][:, 0:1], None, op0=OP.add,
                    )

                def kq_mm_half(ps, wsb, xt, kc):
                    """one kc half into aux psum ps [P, QG]"""
                    for c in range(NDC):
                        nc.tensor.matmul(
                            ps[:],
                            wsb[:, c * HD + kc * P:c * HD + (kc + 1) * P],
                            xt[:, c, :],
                            start=(c == 0), stop=(c == NDC - 1),
                        )

                def v_pass(t):
                    """project value token block t into Vt[t]"""
                    tg, tt = divmod(t, 4)
                    aux = auxps.tile([P, QG], f32, tag="aux", name=f"vp{t}")
                    for c in range(NDC):
                        nc.tensor.matmul(
                            aux[:, 0:HD],
                            xv_t[tg][:, c, tt * P:(tt + 1) * P],
                            wv_sb[:, c * HD:(c + 1) * HD],
                            start=(c == 0), stop=(c == NDC - 1),
                        )
                    nc.vector.tensor_copy(Vt[t][:], aux[:, 0:HD])

                def k_pass_kc(tgj, kc):
                    """K projection of one (token group, kc half) through
                    a score-psum slot: half the slot-steal of a full
                    k_pass, so the exp pipeline bubbles less."""
                    ps = scps.tile([P, 2 * QG], f32, tag="sc",
                                   name=f"kps{tgj}_{kc}")
                    kq_mm_half(ps[:, 0:QG], wk_sb, xk_t[tgj], kc)
                    k_drain(ps[:, 0:QG], tgj, kc)

                def q_pass(tgj, kc):
                    aux = auxps.tile([P, QG], f32, tag="aux",
                                     name=f"qp{tgj}_{kc}")
                    kq_mm_half(aux, wq_sb, xq_t[tgj], kc)
                    q_drain(aux[:], tgj, kc)

                mk_tiles = {}

                def prefetch_mask(qg, kb):
                    mk2 = mpool.tile([P, 2, QG], f16, tag="mk", name="mk")
                    nc.sync.dma_start(
                        mk2[:],
                        maskT.rearrange("(t p) n -> p t n", p=P)[
                            :, kb:kb + 2, qg * QG:(qg + 1) * QG],
                    )
                    mk_tiles[(qg, kb)] = mk2

                def dma_x_half(pool, xdr, tg, half, nm):
                    hc = NDC // 2
                    t = pool.tile([P, hc, QG], f16, tag=nm,
                                  name=f"{nm}{tg}{half}")
                    nc.sync.dma_start(
                        t[:], _xslice(xdr)[:, half * hc:(half + 1) * hc,
                                           tg * QG:(tg + 1) * QG])
                    return t

                def kq_mm_full2(ps, wsb, xta, xtb):
                    """like kq_mm_full but over two half-c tiles, so the
                    projection starts as soon as the first half lands."""
                    hc = NDC // 2
                    for c in range(NDC):
                        xt, cc = (xta, c) if c < hc else (xtb, c - hc)
                        for kc in range(2):
                            nc.tensor.matmul(
                                ps[:, kc * QG:(kc + 1) * QG],
                                wsb[:, c * HD + kc * P:c * HD + (kc + 1) * P],
                                xt[:, cc, :],
                                start=(c == 0), stop=(c == NDC - 1),
                            )

                def _load_w_half(wsb, wdr, kc):
                    nc.sync.dma_start(
                        wsb[:].rearrange("p (c n) -> p c n",
                                         n=HD)[:, :, kc * P:(kc + 1) * P],
                        wdr.rearrange("(c p) n -> p c n",
                                      p=P)[:, :, kc * P:(kc + 1) * P],
                    )

                def kq_mm_half2(ps, wsb, xta, xtb, kc):
                    hc = NDC // 2
                    for c in range(NDC):
                        xt, cc = (xta, c) if c < hc else (xtb, c - hc)
                        nc.tensor.matmul(
                            ps[:, kc * QG:(kc + 1) * QG],
                            wsb[:, c * HD + kc * P:c * HD + (kc + 1) * P],
                            xt[:, cc, :],
                            start=(c == 0), stop=(c == NDC - 1),
                        )

                # ---------- upfront: weights + first token groups ----
                # the first exp needs only the kc0 halves of wk/wq, so
                # stream those first and project per-kc half
                _load_w_half(wk_sb, wkT, 0)
                xk0a = dma_x_half(xkp, xkT, 0, 0, "xk")
                xk0b = dma_x_half(xkp, xkT, 0, 1, "xk")
                _load_w_half(wq_sb, wqT, 0)
                for i in range(2):
                    # tiny loads go on the SWDGE queue so they don't
                    # stall the bulk input stream on the sync queue
                    nc.gpsimd.dma_start(bq_sb[i][:], bq[i * P:(i + 1) * P, :])
                xq0a = dma_x_half(xqp, xqT, 0, 0, "xq")
                xq0b = dma_x_half(xqp, xqT, 0, 1, "xq")
                _load_w_half(wk_sb, wkT, 1)
                _load_w_half(wq_sb, wqT, 1)
                prefetch_mask(0, 0)
                _load_w(wv_sb, wvT)
                dma_x(xvp, xv_t, xvT, 0, "xv")
                prefetch_mask(0, 2)
                dma_x(xkp, xk_t, xkT, 1, "xk")
                dma_x(xvp, xv_t, xvT, 1, "xv")
                for i in range(2):
                    nc.sync.dma_start(wo_sb[i][:], woT[i * P:(i + 1) * P, :])
                nc.gpsimd.memset(ones_sb[:], 1.0)
                nc.gpsimd.memset(ones64[:], 1.0)

                # K tg0, Q tg0 through the big score-psum tiles,
                # kc0 halves first (they gate the first exp)
                psk = scps.tile([P, 2 * QG], f32, tag="sc", name="kps0")
                psq = scps.tile([P, 2 * QG], f32, tag="sc", name="qps0")
                kq_mm_half2(psk, wk_sb, xk0a, xk0b, 0)
                k_drain(psk[:, 0:QG], 0, 0)
                kq_mm_half2(psq, wq_sb, xq0a, xq0b, 0)
                q_drain(psq[:, 0:QG], 0, 0)
                kq_mm_half2(psk, wk_sb, xk0a, xk0b, 1)
                k_drain(psk[:, QG:2 * QG], 0, 1)
                kq_mm_half2(psq, wq_sb, xq0a, xq0b, 1)
                q_drain(psq[:, QG:2 * QG], 0, 1)

                # ---------- attention ----------
                norm_state = {}

                def kb_front(qg, kb):
                    """scores + exp + mask-mul for one key block."""
                    mk = mk_tiles[(qg, kb - kb % 2)][:, kb % 2, :]
                    pa = ppool.tile([P, 4 * QG], f16, tag="pa", name="pa")
                    for pr in range(2):
                        sc = scps.tile([P, 2 * QG], f32, tag="sc", name="sc")
                        for hh in range(2):
                            h = pr * 2 + hh
                            kc, sub = divmod(h, 2)
                            nc.tensor.matmul(
                                sc[:, hh * QG:(hh + 1) * QG],
                                KT[kc][sub * DK:(sub + 1) * DK,
                                       kb * P:(kb + 1) * P],
                                QT[kc][sub * DK:(sub + 1) * DK,
                                       qg * QG:(qg + 1) * QG],
                                start=True, stop=True,
                                tile_position=(sub * DK, 0),
                            )
                        nc.scalar.activation(
                            pa[:, pr * 2 * QG:(pr + 1) * 2 * QG], sc[:],
                            AF.Exp, scale=float(SCALE))
                        # mask per head-pair so PV pair pr can launch as
                        # soon as ITS exp half lands (not both)
                        pav2 = pa[:, pr * 2 * QG:(pr + 1) * 2 * QG]\
                            .rearrange("p (h n) -> p h n", n=QG)
                        nc.vector.tensor_tensor(
                            pav2, pav2,
                            mk.unsqueeze(1).broadcast_to((P, 2, QG)),
                            op=OP.mult,
                        )
                    return pa

                def kb_pv(kb, pa, X01, den):
                    pav4 = pa[:].rearrange("p (h n) -> p h n", n=QG)
                    for pr in range(2):
                        X = X01[pr]
                        for hh in range(2):
                            h = pr * 2 + hh
                            nc.tensor.matmul(
                                X[hh * DK:(hh + 1) * DK, :],
                                Vt[kb][:, h * DK:(h + 1) * DK],
                                pav4[:, h, :],
                                start=(kb == 0), stop=(kb == NKB - 1),
                                tile_position=(0, hh * DK),
                            )
                    # all 4 ones-matmuls back-to-back: they occupy 4
                    # distinct column strips and run concurrently
                    for h in range(HPC):
                        nc.tensor.matmul(
                            den[32 * h:32 * h + 1, :],
                            ones_sb[:, 0:1],
                            pav4[:, h, :],
                            start=(kb == 0), stop=(kb == NKB - 1),
                            tile_position=(0, 32 * h),
                        )

                def rb_broadcast(qg, kc):
                    """replicate denominator-reciprocal rows across 64
                    partitions with two tiny K=1 matmuls into one aux
                    psum bank (the PE drain IS the broadcast)."""
                    xs2, rc16, rbs = norm_state[qg]
                    rb = auxps.tile([P, QG], f32, tag="aux",
                                    name=f"rb{qg}_{kc}")
                    for sub in range(2):
                        h = kc * 2 + sub
                        nc.tensor.matmul(
                            rb[sub * DK:(sub + 1) * DK, :],
                            ones64[32 * h:32 * h + 1, :],
                            rc16[32 * h:32 * h + 1, :],
                            start=True, stop=True,
                            tile_position=(32 * h, sub * DK),
                        )
                    rbs[kc] = rb

                def boundary(qg, X01, den):
                    """drain X psums pairwise, reciprocal of the softmax
                    denominators (cast to f16 so the broadcast matmuls
                    stay off the slow fp32 PE path)."""
                    xs2 = []
                    for kc in range(2):
                        t = xspool.tile([P, QG], f32, tag=f"xs{kc}",
                                        name=f"xs{kc}")
                        nc.vector.tensor_copy(t[:], X01[kc][:])
                        xs2.append(t)
                    rc = rcpool.tile([P, QG], f32, tag="rc", name="rc")
                    nc.vector.reciprocal_approx_fast(rc[0:97, :],
                                                     den[0:97, :])
                    rc16 = rcpool.tile([P, QG], f16, tag="rc16",
                                       name="rc16")
                    nc.vector.tensor_copy(rc16[0:97, :], rc[0:97, :])
                    norm_state[qg] = (xs2, rc16, [None, None])

                def norm_tt(qg, kc):
                    xs2, rc16, rbs = norm_state[qg]
                    nc.vector.tensor_tensor(
                        XN[kc][:, qg * QG:(qg + 1) * QG],
                        xs2[kc][:], rbs[kc][:], op=OP.mult)

                def outproj_step(qg, dm, allow_den=False, scalar_drain=False):
                    # in the tail the score psum banks are free: cycle
                    # through aux/den/sc for a 4-deep drain pipeline
                    pool, tg_ = (auxps, "aux")
                    if allow_den:
                        pool, tg_ = [(auxps, "aux"), (denps, "den"),
                                     (scps, "sc"), (scps, "sc")][dm % 4]
                    ops = pool.tile([P, QG], f32, tag=tg_, name=f"op{dm}")
                    for hd in range(2):
                        nc.tensor.matmul(
                            ops[:],
                            wo_sb[hd][:, dm * P:(dm + 1) * P],
                            XN[hd][:, qg * QG:(qg + 1) * QG],
                            start=(hd == 0), stop=(hd == 1),
                        )
                    ost = ostpool.tile([P, QG], f16, tag="ost", name="ost")
                    if scalar_drain:
                        nc.scalar.copy(ost[:], ops[:])
                    else:
                        nc.vector.tensor_copy(ost[:], ops[:])
                    nc.sync.dma_start(
                        outT[dm * P:(dm + 1) * P, qg * QG:(qg + 1) * QG],
                        ost[:])

                # ---------- global slot stream ----------
                # pv lags front by TWO slots: the PE queue then holds
                # scores(kb+2) BEFORE pv(kb) (which waits on exp+mask of
                # kb), so ScalarE never starves on the
                # mask->pv->den->scores->exp cycle.
                LAG = 3
                pa_hold = {}
                X01 = None
                den = None
                NS = NQG * NKB
                QG0_V = {1: (0, 1), 2: (2,), 3: (3,), 4: (4,), 5: (5,),
                         6: (6,), 7: (7, 8), 8: (9,), 9: (10, 11),
                         10: (12, 13), 11: (14, 15)}
                QG0_K = {2: (1, 0), 3: (1, 1), 5: (2, 0), 6: (2, 1),
                         8: (3, 0), 9: (3, 1)}
                QG0_DMA = {2: ("xk", 2), 3: ("xv", 2), 5: ("xk", 3),
                           6: ("xq", 1), 7: ("xv", 3)}
                for s in range(NS + LAG):
                    if s < NS:
                        qg, kb = divmod(s, NKB)
                        # normalize multiplies go FIRST on the DVE queue
                        # so they free the aux psum bank before the PE
                        # reaches the next rb_broadcast / outproj matmuls
                        if qg > 0:
                            if kb == 4:
                                norm_tt(qg - 1, 0)
                            elif kb == 5:
                                norm_tt(qg - 1, 1)
                        # paced input DMAs (sync-queue order == need order)
                        if qg == 0:
                            if kb in QG0_DMA:
                                nm, tg_i = QG0_DMA[kb]
                                pool = {"xk": xkp, "xq": xqp,
                                        "xv": xvp}[nm]
                                store = {"xk": xk_t, "xq": xq_t,
                                         "xv": xv_t}[nm]
                                xdr = {"xk": xkT, "xq": xqT,
                                       "xv": xvT}[nm]
                                dma_x(pool, store, xdr, tg_i, nm)
                        elif qg in (1, 2) and kb == 0:
                            dma_x(xqp, xq_t, xqT, qg + 1, "xq")
                        if kb % 2 == 0:
                            if kb + 4 < NKB:
                                prefetch_mask(qg, kb + 4)
                            elif qg + 1 < NQG:
                                prefetch_mask(qg + 1, (kb + 4) % NKB)

                        pa_hold[s] = kb_front(qg, kb)

                    if s >= LAG:
                        pqg, pkb = divmod(s - LAG, NKB)
                        if pkb == 0:
                            X01 = [xps.tile([P, QG], f32, tag=f"X{i}",
                                            name=f"X{i}")
                                   for i in range(2)]
                            den = denps.tile([P, QG], f32, tag="den",
                                             name="den")
                        kb_pv(pkb, pa_hold.pop(s - LAG), X01, den)
                        if pkb == NKB - 1:
                            boundary(pqg, X01, den)

                    if s < NS:
                        # denominator broadcasts after this slot's pv so
                        # the PE queue never waits on the reciprocal
                        if qg > 0:
                            if kb == 3:
                                rb_broadcast(qg - 1, 0)
                            elif kb == 4:
                                rb_broadcast(qg - 1, 1)
                        # interleaved output projection of previous group
                        if qg > 0 and 6 <= kb <= 13:
                            outproj_step(qg - 1, kb - 6)

                        # interleaved projections (first query group):
                        # v_pass(t) must complete by slot t+2 (pv lag),
                        # k_pass halves before scores reach key group tg
                        if qg == 0:
                            for t in QG0_V.get(kb, ()):
                                v_pass(t)
                            if kb in QG0_K:
                                k_pass_kc(*QG0_K[kb])
                            if kb in (13, 14):
                                q_pass(1, kb - 13)
                        elif qg in (1, 2) and kb in (13, 14):
                            q_pass(qg + 1, kb - 13)

                # tail: last query group normalize + output projection
                rb_broadcast(NQG - 1, 0)
                norm_tt(NQG - 1, 0)
                rb_broadcast(NQG - 1, 1)
                norm_tt(NQG - 1, 1)
                for dm in range(NDC):
                    outproj_step(NQG - 1, dm, allow_den=True,
                                 scalar_drain=(dm % 2 == 0))

    nc.compile()
    return nc


def _get_nc():
    if "nc" not in _CACHE:
        _CACHE["nc"] = _build_nc()
    return _CACHE["nc"]


def _install_trace_shim():
    """Register the axon NTFF profile hook (dev/test only)."""
    import sys, types
    if "antenv.axon_hooks" in sys.modules:
        return
    try:
        import antenv
        from trn_agent_boot.trn_boot import _ntff_profile_via_ctypes
    except ImportError:
        return
    mod = types.ModuleType("antenv.axon_hooks")
    _hook = [_ntff_profile_via_ctypes("/opt/axon/libaxon_pjrt.so")]
    mod.get_axon_ntff_profile_hook = lambda: _hook[0]
    mod.set_axon_ntff_profile_hook = lambda h: _hook.__setitem__(0, h)
    sys.modules["antenv.axon_hooks"] = mod
    antenv.axon_hooks = mod


def kernel(query, key, value, mask, Wq, bq, Wk, bk, Wv, bv, Wo, bo):
    from concourse.bass_utils import run_bass_kernel_spmd

    query = np.asarray(query, np.float32)
    key = np.asarray(key, np.float32)
    value = np.asarray(value, np.float32)
    mask = np.asarray(mask)
    Wq = np.asarray(Wq, np.float32); bq = np.asarray(bq, np.float32)
    Wk = np.asarray(Wk, np.float32)
    Wv = np.asarray(Wv, np.float32); bv = np.asarray(bv, np.float32)
    Wo = np.asarray(Wo, np.float32); bo = np.asarray(bo, np.float32)

    nc = _get_nc()

    qT = {b: np.ascontiguousarray(query[b].T.astype(np.float16))
          for b in range(B)}
    kT = {b: np.ascontiguousarray(key[b].T.astype(np.float16))
          for b in range(B)}
    vT = {b: np.ascontiguousarray(value[b].T.astype(np.float16))
          for b in range(B)}
    mT = {b: np.ascontiguousarray((mask[b].T == 0).astype(np.float16))
          for b in range(B)}

    in_maps = []
    for c in range(NCORES):
        b, hg = divmod(c, CPB)
        sl = slice(hg * HD, (hg + 1) * HD)
        in_maps.append({
            "xqT": qT[b],
            "xkT": kT[b],
            "xvT": vT[b],
            "wqT": np.ascontiguousarray(Wq[sl].T.astype(np.float16)),
            "wkT": np.ascontiguousarray(Wk[sl].T.astype(np.float16)),
            "wvT": np.ascontiguousarray(Wv[sl].T.astype(np.float16)),
            "woT": np.ascontiguousarray(Wo[:, sl].T.astype(np.float16)),
            "bq": np.ascontiguousarray(bq[sl].reshape(HD, 1)),
            "maskT": mT[b],
        })

    trace = bool(int(os.environ.get("BASS_KERNEL_TRACE", "0")))
    if trace:
        _install_trace_shim()
    res = run_bass_kernel_spmd(nc, in_maps, core_ids=list(range(NCORES)),
                               trace=trace)
    _CACHE["last_perf"] = res

    out = np.zeros((B, S, D), np.float32)
    for c in range(NCORES):
        b = c // CPB
        out[b] += res.results[c]["outT"].T.astype(np.float32)
    out += (Wo @ bv + bo)[None, None, :]
    return out


# revision 44
# speedup vs baseline: 1.0186x; 1.0186x over previous
"""Multi-head attention on 8 Trainium2 NeuronCores.

Sharding: data-parallel over batch (2) x tensor-parallel over heads
(16 heads -> 4 per core). Each core computes QKV projections for its
4 heads, masked softmax attention, and a partial output projection
(row-parallel Wo); the host sums the 4 per-batch partials and folds
in the biases that commute with the linear output projection
(out[b] = sum_partials.T + bo + Wo @ bv).  bk is dropped entirely:
adding bk to K only shifts every query's scores by a per-query
constant, which softmax cancels exactly.

Fully software-pipelined single-pass schedule (v2):
  - ScalarE runs ONLY the softmax exps (the pacer: 2x ~1.15us per key
    block); PSUM drains on DVE, denominator broadcast via SBUF->SBUF
    DMA with a 0-stride partition AP (no GpSimd, no staging copies).
  - The (qg, kb) loops are flattened into one global slot stream so
    the next query group's score matmuls sit in the PE queue BEFORE
    the previous group's last PV/den matmuls (which wait on exp).
    This removes the 3.5us PE bubble per query-group boundary that
    also caused HAM re-throttling.
  - kb_pv orders all 4 den (ones) matmuls after both PV pairs so the
    4 den matmuls run concurrently in 4 distinct PE column strips.
  - Normalize works on head PAIRS: X psum banks are drained with two
    [128,512] copies (heads share a bank), one reciprocal, two
    [128,512] multiplies -- half the DVE ops of per-head normalize.
  - Output projection of query group qg is interleaved into query
    group qg+1's key loop; out is fp16, host accumulates in fp32.
"""

import os
import numpy as np

B, S, D = 2, 2048, 1024
H, DK = 16, 64
NCORES = 8
CPB = 4                 # cores per batch
HPC = H // CPB          # 4 heads per core
HD = HPC * DK           # 256
P = 128
QG = 512                # query block
NQG = S // QG
NKB = S // P            # key blocks
NDC = D // P            # d_in chunks
SCALE = 1.0 / np.sqrt(DK)

_CACHE = {}


def _build_nc():
    import concourse.mybir as mybir
    import concourse.tile as tile
    from concourse import bacc, bass

    f32 = mybir.dt.float32
    f16 = mybir.dt.float16
    AF = mybir.ActivationFunctionType
    OP = mybir.AluOpType

    nc = bacc.Bacc("TRN2", target_bir_lowering=False, debug=False)
    xqT = nc.dram_tensor("xqT", [D, S], f16, kind="ExternalInput")
    xkT = nc.dram_tensor("xkT", [D, S], f16, kind="ExternalInput")
    xvT = nc.dram_tensor("xvT", [D, S], f16, kind="ExternalInput")
    wqT = nc.dram_tensor("wqT", [D, HD], f16, kind="ExternalInput")
    wkT = nc.dram_tensor("wkT", [D, HD], f16, kind="ExternalInput")
    wvT = nc.dram_tensor("wvT", [D, HD], f16, kind="ExternalInput")
    woT = nc.dram_tensor("woT", [HD, D], f16, kind="ExternalInput")
    bq = nc.dram_tensor("bq", [HD, 1], f32, kind="ExternalInput")
    maskT = nc.dram_tensor("maskT", [S, S], f16, kind="ExternalInput")
    outT = nc.dram_tensor("outT", [D, S], f16, kind="ExternalOutput")

    with tile.TileContext(nc) as tc:
        with (
            tc.tile_pool(name="wpool", bufs=1) as wpool,
            tc.tile_pool(name="xkp", bufs=2) as xkp,
            tc.tile_pool(name="xqp", bufs=2) as xqp,
            tc.tile_pool(name="xvp", bufs=2) as xvp,
            tc.tile_pool(name="proj", bufs=1) as proj,
            tc.tile_pool(name="vpool", bufs=1) as vpool,
            tc.tile_pool(name="mpool", bufs=4) as mpool,
            tc.tile_pool(name="ppool", bufs=4) as ppool,
            tc.tile_pool(name="xspool", bufs=2) as xspool,
            tc.tile_pool(name="rcpool", bufs=2) as rcpool,
            tc.tile_pool(name="rbpool", bufs=2) as rbpool,
            tc.tile_pool(name="xnpool", bufs=1) as xnpool,
            tc.tile_pool(name="ostpool", bufs=3) as ostpool,
        ):
            # ---------------- static SBUF tensors ----------------
            wq_sb = wpool.tile([P, NDC * HD], f16, tag="wq")
            wk_sb = wpool.tile([P, NDC * HD], f16, tag="wk")
            wv_sb = wpool.tile([P, NDC * HD], f16, tag="wv")
            wo_sb = [wpool.tile([P, D], f16, tag=f"wo{i}", name=f"wo{i}")
                     for i in range(2)]
            bq_sb = [wpool.tile([P, 1], f32, tag=f"bq{i}", name=f"bqt{i}")
                     for i in range(2)]
            ones_sb = wpool.tile([P, 1], f16, tag="ones")
            ones64 = wpool.tile([P, DK], f16, tag="ones64")

            KT = [proj.tile([P, S], f16, tag=f"KT{i}", name=f"KT{i}")
                  for i in range(2)]
            QT = [proj.tile([P, S], f16, tag=f"QT{i}", name=f"QT{i}")
                  for i in range(2)]
            Vt = [vpool.tile([P, HD], f16, tag=f"V{t}", name=f"Vt{t}")
                  for t in range(NKB)]
            XN = [xnpool.tile([P, S], f16, tag=f"XN{i}", name=f"XN{i}")
                  for i in range(2)]

            def _load_w(wsb, wdr):
                nc.sync.dma_start(
                    wsb[:].rearrange("p (c n) -> p c n", n=HD),
                    wdr.rearrange("(c p) n -> p c n", p=P),
                )

            def _xslice(xdr):
                return xdr.rearrange("(c p) n -> p c n", p=P)

            xk_t, xq_t, xv_t = {}, {}, {}

            def dma_x(pool, store, xdr, tg, nm):
                t = pool.tile([P, NDC, QG], f16, tag=nm, name=f"{nm}{tg}")
                nc.sync.dma_start(
                    t[:], _xslice(xdr)[:, :, tg * QG:(tg + 1) * QG])
                store[tg] = t

            with (
                tc.tile_pool(name="scps", bufs=2, space="PSUM") as scps,
                tc.tile_pool(name="xps", bufs=1, space="PSUM") as xps,
                tc.tile_pool(name="denps", bufs=1, space="PSUM") as denps,
                tc.tile_pool(name="auxps", bufs=1, space="PSUM") as auxps,
            ):
                # ---------- projection helpers ----------
                def kq_mm_full(ps, wsb, xt):
                    """both kc halves of one token group into ps [P, 2*QG]"""
                    for c in range(NDC):
                        for kc in range(2):
                            nc.tensor.matmul(
                                ps[:, kc * QG:(kc + 1) * QG],
                                wsb[:, c * HD + kc * P:c * HD + (kc + 1) * P],
                                xt[:, c, :],
                                start=(c == 0), stop=(c == NDC - 1),
                            )

                def k_drain(src_ap, tg, kc):
                    nc.vector.tensor_copy(
                        KT[kc][:, tg * QG:(tg + 1) * QG], src_ap)

                def q_drain(src_ap, tg, kc):
                    nc.vector.tensor_scalar(
                        QT[kc][:, tg * QG:(tg + 1) * QG],
                        src_ap,
                        bq_sb[kc][:, 0:1], None, op0=OP.add,
                    )

                def kq_mm_half(ps, wsb, xt, kc):
                    """one kc half into aux psum ps [P, QG]"""
                    for c in range(NDC):
                        nc.tensor.matmul(
                            ps[:],
                            wsb[:, c * HD + kc * P:c * HD + (kc + 1) * P],
                            xt[:, c, :],
                            start=(c == 0), stop=(c == NDC - 1),
                        )

                def v_pass(t):
                    """project value token block t into Vt[t]"""
                    tg, tt = divmod(t, 4)
                    aux = auxps.tile([P, QG], f32, tag="aux", name=f"vp{t}")
                    for c in range(NDC):
                        nc.tensor.matmul(
                            aux[:, 0:HD],
                            xv_t[tg][:, c, tt * P:(tt + 1) * P],
                            wv_sb[:, c * HD:(c + 1) * HD],
                            start=(c == 0), stop=(c == NDC - 1),
                        )
                    nc.vector.tensor_copy(Vt[t][:], aux[:, 0:HD])

                # projection drains are deferred to the NEXT slot's start
                # (ahead of the mask TT on the DVE queue) so the psum
                # tile frees before the PE reaches the next consumer --
                # otherwise the PE FIFO stalls on a DVE drain that sits
                # behind a 1.2us mask multiply.
                pend_drain = []

                def k_mm(tgj, kc):
                    ps = scps.tile([P, 2 * QG], f32, tag="sc",
                                   name=f"kps{tgj}_{kc}")
                    kq_mm_half(ps[:, 0:QG], wk_sb, xk_t[tgj], kc)
                    pend_drain.append(("k", ps, tgj, kc))

                def q_mm(tgj, kc):
                    aux = auxps.tile([P, QG], f32, tag="aux",
                                     name=f"qp{tgj}_{kc}")
                    kq_mm_half(aux, wq_sb, xq_t[tgj], kc)
                    pend_drain.append(("q", aux, tgj, kc))

                def flush_drains():
                    while pend_drain:
                        kind, t, tgj, kc = pend_drain.pop(0)
                        if kind == "q":
                            q_drain(t[:], tgj, kc)
                        else:
                            k_drain(t[:, 0:QG], tgj, kc)

                mk_tiles = {}

                def prefetch_mask(qg, kb):
                    mk2 = mpool.tile([P, 2, QG], f16, tag="mk", name="mk")
                    nc.sync.dma_start(
                        mk2[:],
                        maskT.rearrange("(t p) n -> p t n", p=P)[
                            :, kb:kb + 2, qg * QG:(qg + 1) * QG],
                    )
                    mk_tiles[(qg, kb)] = mk2

                def dma_x_half(pool, xdr, tg, half, nm):
                    hc = NDC // 2
                    t = pool.tile([P, hc, QG], f16, tag=nm,
                                  name=f"{nm}{tg}{half}")
                    nc.sync.dma_start(
                        t[:], _xslice(xdr)[:, half * hc:(half + 1) * hc,
                                           tg * QG:(tg + 1) * QG])
                    return t

                def kq_mm_full2(ps, wsb, xta, xtb):
                    """like kq_mm_full but over two half-c tiles, so the
                    projection starts as soon as the first half lands."""
                    hc = NDC // 2
                    for c in range(NDC):
                        xt, cc = (xta, c) if c < hc else (xtb, c - hc)
                        for kc in range(2):
                            nc.tensor.matmul(
                                ps[:, kc * QG:(kc + 1) * QG],
                                wsb[:, c * HD + kc * P:c * HD + (kc + 1) * P],
                                xt[:, cc, :],
                                start=(c == 0), stop=(c == NDC - 1),
                            )

                def _load_w_half(wsb, wdr, kc):
                    nc.sync.dma_start(
                        wsb[:].rearrange("p (c n) -> p c n",
                                         n=HD)[:, :, kc * P:(kc + 1) * P],
                        wdr.rearrange("(c p) n -> p c n",
                                      p=P)[:, :, kc * P:(kc + 1) * P],
                    )

                def kq_mm_half2(ps, wsb, xta, xtb, kc):
                    hc = NDC // 2
                    for c in range(NDC):
                        xt, cc = (xta, c) if c < hc else (xtb, c - hc)
                        nc.tensor.matmul(
                            ps[:, kc * QG:(kc + 1) * QG],
                            wsb[:, c * HD + kc * P:c * HD + (kc + 1) * P],
                            xt[:, cc, :],
                            start=(c == 0), stop=(c == NDC - 1),
                        )

                # ---------- upfront: weights + first token groups ----
                # the first exp needs only the kc0 halves of wk/wq, so
                # stream those first and project per-kc half
                _load_w_half(wk_sb, wkT, 0)
                xk0a = dma_x_half(xkp, xkT, 0, 0, "xk")
                xk0b = dma_x_half(xkp, xkT, 0, 1, "xk")
                _load_w_half(wq_sb, wqT, 0)
                for i in range(2):
                    # tiny loads go on the SWDGE queue so they don't
                    # stall the bulk input stream on the sync queue
                    nc.gpsimd.dma_start(bq_sb[i][:], bq[i * P:(i + 1) * P, :])
                xq0a = dma_x_half(xqp, xqT, 0, 0, "xq")
                xq0b = dma_x_half(xqp, xqT, 0, 1, "xq")
                _load_w_half(wk_sb, wkT, 1)
                _load_w_half(wq_sb, wqT, 1)
                prefetch_mask(0, 0)
                _load_w(wv_sb, wvT)
                dma_x(xvp, xv_t, xvT, 0, "xv")
                prefetch_mask(0, 2)
                dma_x(xkp, xk_t, xkT, 1, "xk")
                dma_x(xvp, xv_t, xvT, 1, "xv")
                for i in range(2):
                    nc.sync.dma_start(wo_sb[i][:], woT[i * P:(i + 1) * P, :])
                nc.gpsimd.memset(ones_sb[:], 1.0)
                nc.gpsimd.memset(ones64[:], 1.0)

                # K tg0, Q tg0 through the big score-psum tiles,
                # kc0 halves first (they gate the first exp)
                psk = scps.tile([P, 2 * QG], f32, tag="sc", name="kps0")
                psq = scps.tile([P, 2 * QG], f32, tag="sc", name="qps0")
                kq_mm_half2(psk, wk_sb, xk0a, xk0b, 0)
                k_drain(psk[:, 0:QG], 0, 0)
                kq_mm_half2(psq, wq_sb, xq0a, xq0b, 0)
                q_drain(psq[:, 0:QG], 0, 0)
                kq_mm_half2(psk, wk_sb, xk0a, xk0b, 1)
                k_drain(psk[:, QG:2 * QG], 0, 1)
                kq_mm_half2(psq, wq_sb, xq0a, xq0b, 1)
                q_drain(psq[:, QG:2 * QG], 0, 1)

                # ---------- attention ----------
                norm_state = {}

                def kb_front(qg, kb):
                    """scores + exp + mask-mul for one key block."""
                    mk = mk_tiles[(qg, kb - kb % 2)][:, kb % 2, :]
                    pa = ppool.tile([P, 4 * QG], f16, tag="pa", name="pa")
                    for pr in range(2):
                        sc = scps.tile([P, 2 * QG], f32, tag="sc", name="sc")
                        for hh in range(2):
                            h = pr * 2 + hh
                            kc, sub = divmod(h, 2)
                            nc.tensor.matmul(
                                sc[:, hh * QG:(hh + 1) * QG],
                                KT[kc][sub * DK:(sub + 1) * DK,
                                       kb * P:(kb + 1) * P],
                                QT[kc][sub * DK:(sub + 1) * DK,
                                       qg * QG:(qg + 1) * QG],
                                start=True, stop=True,
                                tile_position=(sub * DK, 0),
                            )
                        nc.scalar.activation(
                            pa[:, pr * 2 * QG:(pr + 1) * 2 * QG], sc[:],
                            AF.Exp, scale=float(SCALE))
                        # mask per head-pair so PV pair pr can launch as
                        # soon as ITS exp half lands (not both)
                        pav2 = pa[:, pr * 2 * QG:(pr + 1) * 2 * QG]\
                            .rearrange("p (h n) -> p h n", n=QG)
                        nc.vector.tensor_tensor(
                            pav2, pav2,
                            mk.unsqueeze(1).broadcast_to((P, 2, QG)),
                            op=OP.mult,
                        )
                    return pa

                def kb_pv(kb, pa, X01, den):
                    pav4 = pa[:].rearrange("p (h n) -> p h n", n=QG)
                    for pr in range(2):
                        X = X01[pr]
                        for hh in range(2):
                            h = pr * 2 + hh
                            nc.tensor.matmul(
                                X[hh * DK:(hh + 1) * DK, :],
                                Vt[kb][:, h * DK:(h + 1) * DK],
                                pav4[:, h, :],
                                start=(kb == 0), stop=(kb == NKB - 1),
                                tile_position=(0, hh * DK),
                            )
                    # all 4 ones-matmuls back-to-back: they occupy 4
                    # distinct column strips and run concurrently
                    for h in range(HPC):
                        nc.tensor.matmul(
                            den[32 * h:32 * h + 1, :],
                            ones_sb[:, 0:1],
                            pav4[:, h, :],
                            start=(kb == 0), stop=(kb == NKB - 1),
                            tile_position=(0, 32 * h),
                        )

                def rb_broadcast(qg, kc):
                    """replicate denominator-reciprocal rows across 64
                    partitions with two tiny K=1 matmuls into one aux
                    psum bank (the PE drain IS the broadcast)."""
                    xs2, rc16, rbs = norm_state[qg]
                    rb = auxps.tile([P, QG], f32, tag="aux",
                                    name=f"rb{qg}_{kc}")
                    for sub in range(2):
                        h = kc * 2 + sub
                        nc.tensor.matmul(
                            rb[sub * DK:(sub + 1) * DK, :],
                            ones64[32 * h:32 * h + 1, :],
                            rc16[32 * h:32 * h + 1, :],
                            start=True, stop=True,
                            tile_position=(32 * h, sub * DK),
                        )
                    rbs[kc] = rb

                def boundary(qg, X01, den):
                    """drain X psums pairwise, reciprocal of the softmax
                    denominators (cast to f16 so the broadcast matmuls
                    stay off the slow fp32 PE path)."""
                    xs2 = []
                    for kc in range(2):
                        t = xspool.tile([P, QG], f32, tag=f"xs{kc}",
                                        name=f"xs{kc}")
                        nc.vector.tensor_copy(t[:], X01[kc][:])
                        xs2.append(t)
                    rc = rcpool.tile([P, QG], f32, tag="rc", name="rc")
                    nc.vector.reciprocal_approx_fast(rc[0:97, :],
                                                     den[0:97, :])
                    rc16 = rcpool.tile([P, QG], f16, tag="rc16",
                                       name="rc16")
                    nc.vector.tensor_copy(rc16[0:97, :], rc[0:97, :])
                    norm_state[qg] = (xs2, rc16, [None, None])

                def norm_tt(qg, kc):
                    xs2, rc16, rbs = norm_state[qg]
                    nc.vector.tensor_tensor(
                        XN[kc][:, qg * QG:(qg + 1) * QG],
                        xs2[kc][:], rbs[kc][:], op=OP.mult)

                def outproj_step(qg, dm, allow_den=False, scalar_drain=False):
                    # in the tail the score psum banks are free: cycle
                    # through aux/den/sc for a 4-deep drain pipeline
                    pool, tg_ = (auxps, "aux")
                    if allow_den:
                        pool, tg_ = [(auxps, "aux"), (denps, "den"),
                                     (scps, "sc"), (scps, "sc")][dm % 4]
                    ops = pool.tile([P, QG], f32, tag=tg_, name=f"op{dm}")
                    for hd in range(2):
                        nc.tensor.matmul(
                            ops[:],
                            wo_sb[hd][:, dm * P:(dm + 1) * P],
                            XN[hd][:, qg * QG:(qg + 1) * QG],
                            start=(hd == 0), stop=(hd == 1),
                        )
                    ost = ostpool.tile([P, QG], f16, tag="ost", name="ost")
                    if scalar_drain:
                        nc.scalar.copy(ost[:], ops[:])
                    else:
                        nc.vector.tensor_copy(ost[:], ops[:])
                    nc.sync.dma_start(
                        outT[dm * P:(dm + 1) * P, qg * QG:(qg + 1) * QG],
                        ost[:])

                # ---------- global slot stream ----------
                # pv lags front by TWO slots: the PE queue then holds
                # scores(kb+2) BEFORE pv(kb) (which waits on exp+mask of
                # kb), so ScalarE never starves on the
                # mask->pv->den->scores->exp cycle.
                LAG = 2
                pa_hold = {}
                X01 = None
                den = None
                NS = NQG * NKB
                QG0_V = {1: (0, 1), 2: (2,), 3: (3,), 4: (4,), 5: (5,),
                         6: (6,), 7: (7, 8), 8: (9,), 9: (10, 11),
                         10: (12, 13), 11: (14, 15)}
                QG0_K = {2: (1, 0), 3: (1, 1), 5: (2, 0), 6: (2, 1),
                         8: (3, 0), 9: (3, 1)}
                QG0_DMA = {2: ("xk", 2), 3: ("xv", 2), 5: ("xk", 3),
                           6: ("xq", 1), 7: ("xv", 3)}
                for s in range(NS + LAG):
                    flush_drains()
                    if s < NS:
                        qg, kb = divmod(s, NKB)
                        # normalize multiplies go FIRST on the DVE queue
                        # so they free the aux psum bank before the PE
                        # reaches the next rb_broadcast / outproj matmuls
                        if qg > 0:
                            if kb == 4:
                                norm_tt(qg - 1, 0)
                            elif kb == 5:
                                norm_tt(qg - 1, 1)
                        # paced input DMAs (sync-queue order == need order)
                        if qg == 0:
                            if kb in QG0_DMA:
                                nm, tg_i = QG0_DMA[kb]
                                pool = {"xk": xkp, "xq": xqp,
                                        "xv": xvp}[nm]
                                store = {"xk": xk_t, "xq": xq_t,
                                         "xv": xv_t}[nm]
                                xdr = {"xk": xkT, "xq": xqT,
                                       "xv": xvT}[nm]
                                dma_x(pool, store, xdr, tg_i, nm)
                        elif qg in (1, 2) and kb == 0:
                            dma_x(xqp, xq_t, xqT, qg + 1, "xq")
                        if kb % 2 == 0:
                            if kb + 4 < NKB:
                                prefetch_mask(qg, kb + 4)
                            elif qg + 1 < NQG:
                                prefetch_mask(qg + 1, (kb + 4) % NKB)

                        pa_hold[s] = kb_front(qg, kb)

                    if s >= LAG:
                        pqg, pkb = divmod(s - LAG, NKB)
                        if pkb == 0:
                            X01 = [xps.tile([P, QG], f32, tag=f"X{i}",
                                            name=f"X{i}")
                                   for i in range(2)]
                            den = denps.tile([P, QG], f32, tag="den",
                                             name="den")
                        kb_pv(pkb, pa_hold.pop(s - LAG), X01, den)
                        if pkb == NKB - 1:
                            boundary(pqg, X01, den)

                    if s < NS:
                        # denominator broadcasts after this slot's pv so
                        # the PE queue never waits on the reciprocal
                        if qg > 0:
                            if kb == 3:
                                rb_broadcast(qg - 1, 0)
                            elif kb == 4:
                                rb_broadcast(qg - 1, 1)
                        # interleaved output projection of previous group
                        if qg > 0 and 6 <= kb <= 13:
                            outproj_step(qg - 1, kb - 6)

                        # interleaved projections (first query group):
                        # v_pass(t) must complete by slot t+2 (pv lag),
                        # k_pass halves before scores reach key group tg
                        if qg == 0:
                            for t in QG0_V.get(kb, ()):
                                v_pass(t)
                            if kb in QG0_K:
                                k_mm(*QG0_K[kb])
                            if kb in (13, 14):
                                q_mm(1, kb - 13)
                        elif qg in (1, 2) and kb in (13, 14):
                            q_mm(qg + 1, kb - 13)

                # tail: last query group normalize + output projection
                rb_broadcast(NQG - 1, 0)
                norm_tt(NQG - 1, 0)
                rb_broadcast(NQG - 1, 1)
                norm_tt(NQG - 1, 1)
                for dm in range(NDC):
                    outproj_step(NQG - 1, dm, allow_den=True,
                                 scalar_drain=(dm % 2 == 0))

    nc.compile()
    return nc


def _get_nc():
    if "nc" not in _CACHE:
        _CACHE["nc"] = _build_nc()
    return _CACHE["nc"]


def _install_trace_shim():
    """Register the axon NTFF profile hook (dev/test only)."""
    import sys, types
    if "antenv.axon_hooks" in sys.modules:
        return
    try:
        import antenv
        from trn_agent_boot.trn_boot import _ntff_profile_via_ctypes
    except ImportError:
        return
    mod = types.ModuleType("antenv.axon_hooks")
    _hook = [_ntff_profile_via_ctypes("/opt/axon/libaxon_pjrt.so")]
    mod.get_axon_ntff_profile_hook = lambda: _hook[0]
    mod.set_axon_ntff_profile_hook = lambda h: _hook.__setitem__(0, h)
    sys.modules["antenv.axon_hooks"] = mod
    antenv.axon_hooks = mod


def kernel(query, key, value, mask, Wq, bq, Wk, bk, Wv, bv, Wo, bo):
    from concourse.bass_utils import run_bass_kernel_spmd

    query = np.asarray(query, np.float32)
    key = np.asarray(key, np.float32)
    value = np.asarray(value, np.float32)
    mask = np.asarray(mask)
    Wq = np.asarray(Wq, np.float32); bq = np.asarray(bq, np.float32)
    Wk = np.asarray(Wk, np.float32)
    Wv = np.asarray(Wv, np.float32); bv = np.asarray(bv, np.float32)
    Wo = np.asarray(Wo, np.float32); bo = np.asarray(bo, np.float32)

    nc = _get_nc()

    qT = {b: np.ascontiguousarray(query[b].T.astype(np.float16))
          for b in range(B)}
    kT = {b: np.ascontiguousarray(key[b].T.astype(np.float16))
          for b in range(B)}
    vT = {b: np.ascontiguousarray(value[b].T.astype(np.float16))
          for b in range(B)}
    mT = {b: np.ascontiguousarray((mask[b].T == 0).astype(np.float16))
          for b in range(B)}

    in_maps = []
    for c in range(NCORES):
        b, hg = divmod(c, CPB)
        sl = slice(hg * HD, (hg + 1) * HD)
        in_maps.append({
            "xqT": qT[b],
            "xkT": kT[b],
            "xvT": vT[b],
            "wqT": np.ascontiguousarray(Wq[sl].T.astype(np.float16)),
            "wkT": np.ascontiguousarray(Wk[sl].T.astype(np.float16)),
            "wvT": np.ascontiguousarray(Wv[sl].T.astype(np.float16)),
            "woT": np.ascontiguousarray(Wo[:, sl].T.astype(np.float16)),
            "bq": np.ascontiguousarray(bq[sl].reshape(HD, 1)),
            "maskT": mT[b],
        })

    trace = bool(int(os.environ.get("BASS_KERNEL_TRACE", "0")))
    if trace:
        _install_trace_shim()
    res = run_bass_kernel_spmd(nc, in_maps, core_ids=list(range(NCORES)),
                               trace=trace)
    _CACHE["last_perf"] = res

    out = np.zeros((B, S, D), np.float32)
    for c in range(NCORES):
        b = c // CPB
        out[b] += res.results[c]["outT"].T.astype(np.float32)
    out += (Wo @ bv + bo)[None, None, :]
    return out


# revision 45
# speedup vs baseline: 1.0256x; 1.0068x over previous
"""Multi-head attention on 8 Trainium2 NeuronCores.

Sharding: data-parallel over batch (2) x tensor-parallel over heads
(16 heads -> 4 per core). Each core computes QKV projections for its
4 heads, masked softmax attention, and a partial output projection
(row-parallel Wo); the host sums the 4 per-batch partials and folds
in the biases that commute with the linear output projection
(out[b] = sum_partials.T + bo + Wo @ bv).  bk is dropped entirely:
adding bk to K only shifts every query's scores by a per-query
constant, which softmax cancels exactly.

Fully software-pipelined single-pass schedule (v2):
  - ScalarE runs ONLY the softmax exps (the pacer: 2x ~1.15us per key
    block); PSUM drains on DVE, denominator broadcast via SBUF->SBUF
    DMA with a 0-stride partition AP (no GpSimd, no staging copies).
  - The (qg, kb) loops are flattened into one global slot stream so
    the next query group's score matmuls sit in the PE queue BEFORE
    the previous group's last PV/den matmuls (which wait on exp).
    This removes the 3.5us PE bubble per query-group boundary that
    also caused HAM re-throttling.
  - kb_pv orders all 4 den (ones) matmuls after both PV pairs so the
    4 den matmuls run concurrently in 4 distinct PE column strips.
  - Normalize works on head PAIRS: X psum banks are drained with two
    [128,512] copies (heads share a bank), one reciprocal, two
    [128,512] multiplies -- half the DVE ops of per-head normalize.
  - Output projection of query group qg is interleaved into query
    group qg+1's key loop; out is fp16, host accumulates in fp32.
"""

import os
import numpy as np

B, S, D = 2, 2048, 1024
H, DK = 16, 64
NCORES = 8
CPB = 4                 # cores per batch
HPC = H // CPB          # 4 heads per core
HD = HPC * DK           # 256
P = 128
QG = 512                # query block
NQG = S // QG
NKB = S // P            # key blocks
NDC = D // P            # d_in chunks
SCALE = 1.0 / np.sqrt(DK)

_CACHE = {}


def _build_nc():
    import concourse.mybir as mybir
    import concourse.tile as tile
    from concourse import bacc, bass

    f32 = mybir.dt.float32
    f16 = mybir.dt.float16
    AF = mybir.ActivationFunctionType
    OP = mybir.AluOpType

    nc = bacc.Bacc("TRN2", target_bir_lowering=False, debug=False)
    xqT = nc.dram_tensor("xqT", [D, S], f16, kind="ExternalInput")
    xkT = nc.dram_tensor("xkT", [D, S], f16, kind="ExternalInput")
    xvT = nc.dram_tensor("xvT", [D, S], f16, kind="ExternalInput")
    wqT = nc.dram_tensor("wqT", [D, HD], f16, kind="ExternalInput")
    wkT = nc.dram_tensor("wkT", [D, HD], f16, kind="ExternalInput")
    wvT = nc.dram_tensor("wvT", [D, HD], f16, kind="ExternalInput")
    woT = nc.dram_tensor("woT", [HD, D], f16, kind="ExternalInput")
    bq = nc.dram_tensor("bq", [HD, 1], f32, kind="ExternalInput")
    maskT = nc.dram_tensor("maskT", [S, S], f16, kind="ExternalInput")
    outT = nc.dram_tensor("outT", [D, S], f16, kind="ExternalOutput")

    with tile.TileContext(nc) as tc:
        with (
            tc.tile_pool(name="wpool", bufs=1) as wpool,
            tc.tile_pool(name="xkp", bufs=2) as xkp,
            tc.tile_pool(name="xqp", bufs=2) as xqp,
            tc.tile_pool(name="xvp", bufs=2) as xvp,
            tc.tile_pool(name="proj", bufs=1) as proj,
            tc.tile_pool(name="vpool", bufs=1) as vpool,
            tc.tile_pool(name="mpool", bufs=4) as mpool,
            tc.tile_pool(name="ppool", bufs=4) as ppool,
            tc.tile_pool(name="xspool", bufs=2) as xspool,
            tc.tile_pool(name="rcpool", bufs=2) as rcpool,
            tc.tile_pool(name="rbpool", bufs=2) as rbpool,
            tc.tile_pool(name="xnpool", bufs=1) as xnpool,
            tc.tile_pool(name="ostpool", bufs=3) as ostpool,
        ):
            # ---------------- static SBUF tensors ----------------
            wq_sb = wpool.tile([P, NDC * HD], f16, tag="wq")
            wk_sb = wpool.tile([P, NDC * HD], f16, tag="wk")
            wv_sb = wpool.tile([P, NDC * HD], f16, tag="wv")
            wo_sb = [wpool.tile([P, D], f16, tag=f"wo{i}", name=f"wo{i}")
                     for i in range(2)]
            bq_sb = [wpool.tile([P, 1], f32, tag=f"bq{i}", name=f"bqt{i}")
                     for i in range(2)]
            ones_sb = wpool.tile([P, 1], f16, tag="ones")
            ones64 = wpool.tile([P, DK], f16, tag="ones64")

            KT = [proj.tile([P, S], f16, tag=f"KT{i}", name=f"KT{i}")
                  for i in range(2)]
            QT = [proj.tile([P, S], f16, tag=f"QT{i}", name=f"QT{i}")
                  for i in range(2)]
            Vt = [vpool.tile([P, HD], f16, tag=f"V{t}", name=f"Vt{t}")
                  for t in range(NKB)]
            XN = [xnpool.tile([P, S], f16, tag=f"XN{i}", name=f"XN{i}")
                  for i in range(2)]

            def _load_w(wsb, wdr):
                nc.sync.dma_start(
                    wsb[:].rearrange("p (c n) -> p c n", n=HD),
                    wdr.rearrange("(c p) n -> p c n", p=P),
                )

            def _xslice(xdr):
                return xdr.rearrange("(c p) n -> p c n", p=P)

            xk_t, xq_t, xv_t = {}, {}, {}

            def dma_x(pool, store, xdr, tg, nm):
                t = pool.tile([P, NDC, QG], f16, tag=nm, name=f"{nm}{tg}")
                nc.sync.dma_start(
                    t[:], _xslice(xdr)[:, :, tg * QG:(tg + 1) * QG])
                store[tg] = t

            with (
                tc.tile_pool(name="scps", bufs=2, space="PSUM") as scps,
                tc.tile_pool(name="xps", bufs=1, space="PSUM") as xps,
                tc.tile_pool(name="denps", bufs=1, space="PSUM") as denps,
                tc.tile_pool(name="auxps", bufs=1, space="PSUM") as auxps,
            ):
                # ---------- projection helpers ----------
                def kq_mm_full(ps, wsb, xt):
                    """both kc halves of one token group into ps [P, 2*QG]"""
                    for c in range(NDC):
                        for kc in range(2):
                            nc.tensor.matmul(
                                ps[:, kc * QG:(kc + 1) * QG],
                                wsb[:, c * HD + kc * P:c * HD + (kc + 1) * P],
                                xt[:, c, :],
                                start=(c == 0), stop=(c == NDC - 1),
                            )

                def k_drain(src_ap, tg, kc):
                    nc.vector.tensor_copy(
                        KT[kc][:, tg * QG:(tg + 1) * QG], src_ap)

                def q_drain(src_ap, tg, kc):
                    nc.vector.tensor_scalar(
                        QT[kc][:, tg * QG:(tg + 1) * QG],
                        src_ap,
                        bq_sb[kc][:, 0:1], None, op0=OP.add,
                    )

                def kq_mm_half(ps, wsb, xt, kc):
                    """one kc half into aux psum ps [P, QG]"""
                    for c in range(NDC):
                        nc.tensor.matmul(
                            ps[:],
                            wsb[:, c * HD + kc * P:c * HD + (kc + 1) * P],
                            xt[:, c, :],
                            start=(c == 0), stop=(c == NDC - 1),
                        )

                def v_pass(t):
                    """project value token block t into Vt[t]"""
                    tg, tt = divmod(t, 4)
                    aux = auxps.tile([P, QG], f32, tag="aux", name=f"vp{t}")
                    for c in range(NDC):
                        nc.tensor.matmul(
                            aux[:, 0:HD],
                            xv_t[tg][:, c, tt * P:(tt + 1) * P],
                            wv_sb[:, c * HD:(c + 1) * HD],
                            start=(c == 0), stop=(c == NDC - 1),
                        )
                    nc.vector.tensor_copy(Vt[t][:], aux[:, 0:HD])

                def k_pass_kc(tgj, kc):
                    """K projection of one (token group, kc half) through
                    a score-psum slot: half the slot-steal of a full
                    k_pass, so the exp pipeline bubbles less."""
                    ps = scps.tile([P, 2 * QG], f32, tag="sc",
                                   name=f"kps{tgj}_{kc}")
                    kq_mm_half(ps[:, 0:QG], wk_sb, xk_t[tgj], kc)
                    k_drain(ps[:, 0:QG], tgj, kc)

                def q_pass(tgj, kc):
                    aux = auxps.tile([P, QG], f32, tag="aux",
                                     name=f"qp{tgj}_{kc}")
                    kq_mm_half(aux, wq_sb, xq_t[tgj], kc)
                    q_drain(aux[:], tgj, kc)

                mk_tiles = {}

                def prefetch_mask(qg, kb):
                    mk2 = mpool.tile([P, 2, QG], f16, tag="mk", name="mk")
                    nc.sync.dma_start(
                        mk2[:],
                        maskT.rearrange("(t p) n -> p t n", p=P)[
                            :, kb:kb + 2, qg * QG:(qg + 1) * QG],
                    )
                    mk_tiles[(qg, kb)] = mk2

                def dma_x_half(pool, xdr, tg, half, nm):
                    hc = NDC // 2
                    t = pool.tile([P, hc, QG], f16, tag=nm,
                                  name=f"{nm}{tg}{half}")
                    nc.sync.dma_start(
                        t[:], _xslice(xdr)[:, half * hc:(half + 1) * hc,
                                           tg * QG:(tg + 1) * QG])
                    return t

                def kq_mm_full2(ps, wsb, xta, xtb):
                    """like kq_mm_full but over two half-c tiles, so the
                    projection starts as soon as the first half lands."""
                    hc = NDC // 2
                    for c in range(NDC):
                        xt, cc = (xta, c) if c < hc else (xtb, c - hc)
                        for kc in range(2):
                            nc.tensor.matmul(
                                ps[:, kc * QG:(kc + 1) * QG],
                                wsb[:, c * HD + kc * P:c * HD + (kc + 1) * P],
                                xt[:, cc, :],
                                start=(c == 0), stop=(c == NDC - 1),
                            )

                def _load_w_half(wsb, wdr, kc):
                    nc.sync.dma_start(
                        wsb[:].rearrange("p (c n) -> p c n",
                                         n=HD)[:, :, kc * P:(kc + 1) * P],
                        wdr.rearrange("(c p) n -> p c n",
                                      p=P)[:, :, kc * P:(kc + 1) * P],
                    )

                def kq_mm_half2(ps, wsb, xta, xtb, kc):
                    hc = NDC // 2
                    for c in range(NDC):
                        xt, cc = (xta, c) if c < hc else (xtb, c - hc)
                        nc.tensor.matmul(
                            ps[:, kc * QG:(kc + 1) * QG],
                            wsb[:, c * HD + kc * P:c * HD + (kc + 1) * P],
                            xt[:, cc, :],
                            start=(c == 0), stop=(c == NDC - 1),
                        )

                # ---------- upfront: weights + first token groups ----
                # the first exp needs only the kc0 halves of wk/wq, so
                # stream those first and project per-kc half
                _load_w_half(wk_sb, wkT, 0)
                xk0a = dma_x_half(xkp, xkT, 0, 0, "xk")
                xk0b = dma_x_half(xkp, xkT, 0, 1, "xk")
                _load_w_half(wq_sb, wqT, 0)
                for i in range(2):
                    # tiny loads go on the SWDGE queue so they don't
                    # stall the bulk input stream on the sync queue
                    nc.gpsimd.dma_start(bq_sb[i][:], bq[i * P:(i + 1) * P, :])
                xq0a = dma_x_half(xqp, xqT, 0, 0, "xq")
                xq0b = dma_x_half(xqp, xqT, 0, 1, "xq")
                _load_w_half(wk_sb, wkT, 1)
                _load_w_half(wq_sb, wqT, 1)
                prefetch_mask(0, 0)
                _load_w(wv_sb, wvT)
                dma_x(xvp, xv_t, xvT, 0, "xv")
                prefetch_mask(0, 2)
                dma_x(xkp, xk_t, xkT, 1, "xk")
                dma_x(xvp, xv_t, xvT, 1, "xv")
                for i in range(2):
                    nc.sync.dma_start(wo_sb[i][:], woT[i * P:(i + 1) * P, :])
                nc.gpsimd.memset(ones_sb[:], 1.0)
                nc.gpsimd.memset(ones64[:], 1.0)

                # K tg0, Q tg0 through the big score-psum tiles,
                # kc0 halves first (they gate the first exp)
                psk = scps.tile([P, 2 * QG], f32, tag="sc", name="kps0")
                psq = scps.tile([P, 2 * QG], f32, tag="sc", name="qps0")
                kq_mm_half2(psk, wk_sb, xk0a, xk0b, 0)
                k_drain(psk[:, 0:QG], 0, 0)
                kq_mm_half2(psq, wq_sb, xq0a, xq0b, 0)
                q_drain(psq[:, 0:QG], 0, 0)
                kq_mm_half2(psk, wk_sb, xk0a, xk0b, 1)
                k_drain(psk[:, QG:2 * QG], 0, 1)
                kq_mm_half2(psq, wq_sb, xq0a, xq0b, 1)
                q_drain(psq[:, QG:2 * QG], 0, 1)

                # ---------- attention ----------
                norm_state = {}

                def kb_front(qg, kb):
                    """scores + exp + mask-mul for one key block."""
                    mk = mk_tiles[(qg, kb - kb % 2)][:, kb % 2, :]
                    pa = ppool.tile([P, 4 * QG], f16, tag="pa", name="pa")
                    for pr in range(2):
                        sc = scps.tile([P, 2 * QG], f32, tag="sc", name="sc")
                        for hh in range(2):
                            h = pr * 2 + hh
                            kc, sub = divmod(h, 2)
                            nc.tensor.matmul(
                                sc[:, hh * QG:(hh + 1) * QG],
                                KT[kc][sub * DK:(sub + 1) * DK,
                                       kb * P:(kb + 1) * P],
                                QT[kc][sub * DK:(sub + 1) * DK,
                                       qg * QG:(qg + 1) * QG],
                                start=True, stop=True,
                                tile_position=(sub * DK, 0),
                            )
                        nc.scalar.activation(
                            pa[:, pr * 2 * QG:(pr + 1) * 2 * QG], sc[:],
                            AF.Exp, scale=float(SCALE))
                        # mask per head-pair so PV pair pr can launch as
                        # soon as ITS exp half lands (not both)
                        pav2 = pa[:, pr * 2 * QG:(pr + 1) * 2 * QG]\
                            .rearrange("p (h n) -> p h n", n=QG)
                        nc.vector.tensor_tensor(
                            pav2, pav2,
                            mk.unsqueeze(1).broadcast_to((P, 2, QG)),
                            op=OP.mult,
                        )
                    return pa

                def kb_pv(kb, pa, X01, den):
                    pav4 = pa[:].rearrange("p (h n) -> p h n", n=QG)
                    for pr in range(2):
                        X = X01[pr]
                        for hh in range(2):
                            h = pr * 2 + hh
                            nc.tensor.matmul(
                                X[hh * DK:(hh + 1) * DK, :],
                                Vt[kb][:, h * DK:(h + 1) * DK],
                                pav4[:, h, :],
                                start=(kb == 0), stop=(kb == NKB - 1),
                                tile_position=(0, hh * DK),
                            )
                    # all 4 ones-matmuls back-to-back: they occupy 4
                    # distinct column strips and run concurrently
                    for h in range(HPC):
                        nc.tensor.matmul(
                            den[32 * h:32 * h + 1, :],
                            ones_sb[:, 0:1],
                            pav4[:, h, :],
                            start=(kb == 0), stop=(kb == NKB - 1),
                            tile_position=(0, 32 * h),
                        )

                def rb_broadcast(qg, kc):
                    """replicate denominator-reciprocal rows across 64
                    partitions with two tiny K=1 matmuls into one aux
                    psum bank (the PE drain IS the broadcast)."""
                    xs2, rc16, rbs = norm_state[qg]
                    rb = auxps.tile([P, QG], f32, tag="aux",
                                    name=f"rb{qg}_{kc}")
                    for sub in range(2):
                        h = kc * 2 + sub
                        nc.tensor.matmul(
                            rb[sub * DK:(sub + 1) * DK, :],
                            ones64[32 * h:32 * h + 1, :],
                            rc16[32 * h:32 * h + 1, :],
                            start=True, stop=True,
                            tile_position=(32 * h, sub * DK),
                        )
                    rbs[kc] = rb

                def boundary(qg, X01, den):
                    """drain X psums pairwise, reciprocal of the softmax
                    denominators (cast to f16 so the broadcast matmuls
                    stay off the slow fp32 PE path)."""
                    xs2 = []
                    for kc in range(2):
                        t = xspool.tile([P, QG], f32, tag=f"xs{kc}",
                                        name=f"xs{kc}")
                        nc.vector.tensor_copy(t[:], X01[kc][:])
                        xs2.append(t)
                    rc = rcpool.tile([P, QG], f32, tag="rc", name="rc")
                    nc.vector.reciprocal_approx_fast(rc[0:97, :],
                                                     den[0:97, :])
                    rc16 = rcpool.tile([P, QG], f16, tag="rc16",
                                       name="rc16")
                    nc.vector.tensor_copy(rc16[0:97, :], rc[0:97, :])
                    norm_state[qg] = (xs2, rc16, [None, None])

                def norm_tt(qg, kc):
                    xs2, rc16, rbs = norm_state[qg]
                    nc.vector.tensor_tensor(
                        XN[kc][:, qg * QG:(qg + 1) * QG],
                        xs2[kc][:], rbs[kc][:], op=OP.mult)

                def outproj_step(qg, dm, allow_den=False, scalar_drain=False):
                    # in the tail the score psum banks are free: cycle
                    # through aux/den/sc for a 4-deep drain pipeline
                    pool, tg_ = (auxps, "aux")
                    if allow_den:
                        pool, tg_ = [(auxps, "aux"), (denps, "den"),
                                     (scps, "sc"), (scps, "sc")][dm % 4]
                    ops = pool.tile([P, QG], f32, tag=tg_, name=f"op{dm}")
                    for hd in range(2):
                        nc.tensor.matmul(
                            ops[:],
                            wo_sb[hd][:, dm * P:(dm + 1) * P],
                            XN[hd][:, qg * QG:(qg + 1) * QG],
                            start=(hd == 0), stop=(hd == 1),
                        )
                    ost = ostpool.tile([P, QG], f16, tag="ost", name="ost")
                    if scalar_drain:
                        nc.scalar.copy(ost[:], ops[:])
                    else:
                        nc.vector.tensor_copy(ost[:], ops[:])
                    nc.sync.dma_start(
                        outT[dm * P:(dm + 1) * P, qg * QG:(qg + 1) * QG],
                        ost[:])

                # ---------- global slot stream ----------
                # pv lags front by TWO slots: the PE queue then holds
                # scores(kb+2) BEFORE pv(kb) (which waits on exp+mask of
                # kb), so ScalarE never starves on the
                # mask->pv->den->scores->exp cycle.
                LAG = 2
                pa_hold = {}
                X01 = None
                den = None
                NS = NQG * NKB
                QG0_V = {1: (0, 1), 2: (2,), 3: (3,), 4: (4,), 5: (5,),
                         6: (6,), 7: (7, 8), 8: (9,), 9: (10, 11),
                         10: (12, 13), 11: (14, 15)}
                QG0_K = {2: (1, 0), 3: (1, 1), 5: (2, 0), 6: (2, 1),
                         8: (3, 0), 9: (3, 1)}
                QG0_DMA = {2: ("xk", 2), 3: ("xv", 2), 5: ("xk", 3),
                           6: ("xq", 1), 7: ("xv", 3)}
                for s in range(NS + LAG):
                    if s < NS:
                        qg, kb = divmod(s, NKB)
                        # normalize multiplies go FIRST on the DVE queue
                        # so they free the aux psum bank before the PE
                        # reaches the next rb_broadcast / outproj matmuls
                        if qg > 0:
                            if kb == 4:
                                norm_tt(qg - 1, 0)
                            elif kb == 5:
                                norm_tt(qg - 1, 1)
                        # paced input DMAs (sync-queue order == need order)
                        if qg == 0:
                            if kb in QG0_DMA:
                                nm, tg_i = QG0_DMA[kb]
                                pool = {"xk": xkp, "xq": xqp,
                                        "xv": xvp}[nm]
                                store = {"xk": xk_t, "xq": xq_t,
                                         "xv": xv_t}[nm]
                                xdr = {"xk": xkT, "xq": xqT,
                                       "xv": xvT}[nm]
                                dma_x(pool, store, xdr, tg_i, nm)
                        elif qg in (1, 2) and kb == 0:
                            dma_x(xqp, xq_t, xqT, qg + 1, "xq")
                        if kb % 2 == 0:
                            if kb + 4 < NKB:
                                prefetch_mask(qg, kb + 4)
                            elif qg + 1 < NQG:
                                prefetch_mask(qg + 1, (kb + 4) % NKB)

                        pa_hold[s] = kb_front(qg, kb)

                    if s >= LAG:
                        pqg, pkb = divmod(s - LAG, NKB)
                        if pkb == 0:
                            X01 = [xps.tile([P, QG], f32, tag=f"X{i}",
                                            name=f"X{i}")
                                   for i in range(2)]
                            den = denps.tile([P, QG], f32, tag="den",
                                             name="den")
                        kb_pv(pkb, pa_hold.pop(s - LAG), X01, den)
                        if pkb == NKB - 1:
                            boundary(pqg, X01, den)

                    if s < NS:
                        # denominator broadcasts after this slot's pv so
                        # the PE queue never waits on the reciprocal
                        if qg > 0:
                            if kb == 3:
                                rb_broadcast(qg - 1, 0)
                            elif kb == 4:
                                rb_broadcast(qg - 1, 1)
                        # interleaved output projection of previous group
                        if qg > 0 and 6 <= kb <= 13:
                            outproj_step(qg - 1, kb - 6)

                        # interleaved projections (first query group):
                        # v_pass(t) must complete by slot t+2 (pv lag),
                        # k_pass halves before scores reach key group tg
                        if qg == 0:
                            for t in QG0_V.get(kb, ()):
                                v_pass(t)
                            if kb in QG0_K:
                                k_pass_kc(*QG0_K[kb])
                            if kb in (13, 14):
                                q_pass(1, kb - 13)
                        elif qg in (1, 2) and kb in (13, 14):
                            q_pass(qg + 1, kb - 13)

                # tail: last query group normalize + output projection
                rb_broadcast(NQG - 1, 0)
                norm_tt(NQG - 1, 0)
                rb_broadcast(NQG - 1, 1)
                norm_tt(NQG - 1, 1)
                for dm in range(NDC):
                    outproj_step(NQG - 1, dm, allow_den=True,
                                 scalar_drain=(dm % 2 == 0))

    nc.compile()
    return nc


def _get_nc():
    if "nc" not in _CACHE:
        _CACHE["nc"] = _build_nc()
    return _CACHE["nc"]


def _install_trace_shim():
    """Register the axon NTFF profile hook (dev/test only)."""
    import sys, types
    if "antenv.axon_hooks" in sys.modules:
        return
    try:
        import antenv
        from trn_agent_boot.trn_boot import _ntff_profile_via_ctypes
    except ImportError:
        return
    mod = types.ModuleType("antenv.axon_hooks")
    _hook = [_ntff_profile_via_ctypes("/opt/axon/libaxon_pjrt.so")]
    mod.get_axon_ntff_profile_hook = lambda: _hook[0]
    mod.set_axon_ntff_profile_hook = lambda h: _hook.__setitem__(0, h)
    sys.modules["antenv.axon_hooks"] = mod
    antenv.axon_hooks = mod


def kernel(query, key, value, mask, Wq, bq, Wk, bk, Wv, bv, Wo, bo):
    from concourse.bass_utils import run_bass_kernel_spmd

    query = np.asarray(query, np.float32)
    key = np.asarray(key, np.float32)
    value = np.asarray(value, np.float32)
    mask = np.asarray(mask)
    Wq = np.asarray(Wq, np.float32); bq = np.asarray(bq, np.float32)
    Wk = np.asarray(Wk, np.float32)
    Wv = np.asarray(Wv, np.float32); bv = np.asarray(bv, np.float32)
    Wo = np.asarray(Wo, np.float32); bo = np.asarray(bo, np.float32)

    nc = _get_nc()

    qT = {b: np.ascontiguousarray(query[b].T.astype(np.float16))
          for b in range(B)}
    kT = {b: np.ascontiguousarray(key[b].T.astype(np.float16))
          for b in range(B)}
    vT = {b: np.ascontiguousarray(value[b].T.astype(np.float16))
          for b in range(B)}
    mT = {b: np.ascontiguousarray((mask[b].T == 0).astype(np.float16))
          for b in range(B)}

    in_maps = []
    for c in range(NCORES):
        b, hg = divmod(c, CPB)
        sl = slice(hg * HD, (hg + 1) * HD)
        in_maps.append({
            "xqT": qT[b],
            "xkT": kT[b],
            "xvT": vT[b],
            "wqT": np.ascontiguousarray(Wq[sl].T.astype(np.float16)),
            "wkT": np.ascontiguousarray(Wk[sl].T.astype(np.float16)),
            "wvT": np.ascontiguousarray(Wv[sl].T.astype(np.float16)),
            "woT": np.ascontiguousarray(Wo[:, sl].T.astype(np.float16)),
            "bq": np.ascontiguousarray(bq[sl].reshape(HD, 1)),
            "maskT": mT[b],
        })

    trace = bool(int(os.environ.get("BASS_KERNEL_TRACE", "0")))
    if trace:
        _install_trace_shim()
    res = run_bass_kernel_spmd(nc, in_maps, core_ids=list(range(NCORES)),
                               trace=trace)
    _CACHE["last_perf"] = res

    out = np.zeros((B, S, D), np.float32)
    for c in range(NCORES):
        b = c // CPB
        out[b] += res.results[c]["outT"].T.astype(np.float32)
    out += (Wo @ bv + bo)[None, None, :]
    return out
